# Initial kernel scaffold
#
"""Multi-head attention (B=1, S=4096, dim=1024, 16 heads x 64) on 8 NeuronCores.

Sharding: tensor-parallel over heads. Core c computes heads {2c, 2c+1}:
  - Q/K/V projections for its 128 qkv-dims (x is replicated),
  - full attention for its 2 heads (flash-style, S^T layout, softmax
    denominator via an appended ones-column in the AV matmul),
  - its partial out-projection y_c = attn_out_c @ Wo[c*128:(c+1)*128, :].
Host unshards by summing the 8 partials and adding bo.

Matmul operands are fp16 (all intermediates are small-range; rel err
~3e-3); accumulation is fp32 in PSUM and softmax runs in fp32. x is
transposed by the DMA xbar engine. The two heads' K=64 score matmuls run
concurrently on disjoint PE quadrant rows. The first attention stripe is
emitted interleaved with the projection loop (aligned on ks-blocks) so
the PE/ACT pipelines overlap across the phases.
"""

import sys

sys.path.insert(0, "/opt/trn_rl_repo")

import numpy as np

import concourse.bass as bass
import concourse.mybir as mybir
import concourse.tile as tile
from concourse import bacc
from concourse.bass_utils import run_bass_kernel_spmd

F32 = mybir.dt.float32
F16 = mybir.dt.float16
AF = mybir.ActivationFunctionType

S = 4096          # sequence length
DIM = 1024        # model dim
NH = 16           # total heads
DK = 64           # head dim (= DV)
NCORES = 8
HPC = NH // NCORES          # heads per core (2)
DPC = HPC * DK              # qkv dims per core (128)
SCALE = DK ** -0.5

ST = S // 128               # 32 seq tiles of 128
KT = DIM // 128             # 8 contraction tiles
QW = 512                    # q-stripe width for attention (per head)
NT = S // QW                # 8 q-stripes


def build_bass():
    nc = bacc.Bacc(None)

    xt_in = nc.declare_dram_parameter("xt", [DIM, S], F16, isOutput=False)
    wq = nc.declare_dram_parameter("wq", [DIM, DPC], F16, isOutput=False)
    wk = nc.declare_dram_parameter("wk", [DIM, DPC], F16, isOutput=False)
    wv = nc.declare_dram_parameter("wv", [DIM, DPC], F16, isOutput=False)
    bq = nc.declare_dram_parameter("bq", [DPC, 1], F32, isOutput=False)
    bk = nc.declare_dram_parameter("bk", [DPC, 1], F32, isOutput=False)
    bv = nc.declare_dram_parameter("bv", [DPC, 1], F32, isOutput=False)
    wo = nc.declare_dram_parameter("wo", [DPC, DIM], F16, isOutput=False)
    y = nc.declare_dram_parameter("y", [S, DIM], F32, isOutput=True)

    with tile.TileContext(nc) as tc:
        with (
            tc.tile_pool(name="const", bufs=1) as const,
            tc.tile_pool(name="persist", bufs=1) as persist,
            tc.tile_pool(name="work", bufs=2) as work,
            tc.tile_pool(name="pexp", bufs=4) as pexp,
            tc.tile_pool(name="dram", bufs=2, space="DRAM") as dram,
        ):
            # ---- constants / weights ----
            from concourse.masks import make_identity

            ident_f = const.tile([128, 128], F32)
            make_identity(nc, ident_f)
            ident = const.tile([128, 128], F16)
            nc.vector.tensor_copy(ident[:], ident_f[:])
            ones_f = const.tile([128, 1], F32)
            nc.vector.memset(ones_f[:], 1.0)

            # dense PE warmup: trips the HAM activity window to full
            # clock and keeps the array busy until the first projections
            with tc.tile_pool(name="psumw", bufs=2, space="PSUM") as psumw:
                for _w in range(160):
                    wt = psumw.tile([128, 128], F32, tag="warm")
                    nc.tensor.matmul(wt[:], ident[:], ident[:],
                                     start=True, stop=True)

            # ---- persistent activations ----
            xT = persist.tile([128, KT, S], F16)      # x^T
            qT = persist.tile([DPC, S], F16)          # Q^T: [d', s]
            kT = persist.tile([DPC, S], F16)          # K^T: [d', s]
            v_nat = persist.tile([128, ST, 2 * (DK + 1)], F16)
            uT = persist.tile([DPC, S], F16)          # normalized attn out^T

            # x^T comes pre-transposed from the host; plain contiguous
            # loads, j-major so early seq blocks land first. First chunk +
            # projection weights go ahead of everything else.
            xt_r = xt_in.rearrange("(kt p) s -> p kt s", p=128)
            nc.sync.dma_start(xT[:, :, 0:1024], xt_r[:, :, 0:1024])
            wq_sb = const.tile([128, KT, DPC], F16)
            wk_sb = const.tile([128, KT, DPC], F16)
            wv_sb = const.tile([128, KT, DPC], F16)
            nc.sync.dma_start(wq_sb[:], wq.rearrange("(kt p) d -> p kt d", p=128))
            nc.sync.dma_start(wk_sb[:], wk.rearrange("(kt p) d -> p kt d", p=128))
            nc.sync.dma_start(wv_sb[:], wv.rearrange("(kt p) d -> p kt d", p=128))
            bq_sb = const.tile([DPC, 1], F32)
            bk_sb = const.tile([DPC, 1], F32)
            bv_sb = const.tile([DPC, 1], F32)
            nc.sync.dma_start(bq_sb[:], bq[:])
            nc.sync.dma_start(bk_sb[:], bk[:])
            nc.sync.dma_start(bv_sb[:], bv[:])
            for jh in range(1, 4):
                nc.sync.dma_start(
                    xT[:, :, jh * 1024:(jh + 1) * 1024],
                    xt_r[:, :, jh * 1024:(jh + 1) * 1024],
                )
            wo_sb = const.tile([DPC, DIM], F16)
            nc.sync.dma_start(wo_sb[:], wo[:])

            for st in range(ST):
                nc.vector.tensor_copy(v_nat[:, st, DK:DK + 1], ones_f[:])
                nc.vector.tensor_copy(v_nat[:, st, 2 * DK + 1:], ones_f[:])

            with tc.tile_pool(name="psum12", bufs=1, space="PSUM") as psum:

                def proj_block(j):
                    """Q/K/V projections + V transpose for seq block j."""
                    sl = slice(j * 512, (j + 1) * 512)
                    for w_sb, b_sb, dst in (
                        (wq_sb, bq_sb, qT),
                        (wk_sb, bk_sb, kT),
                        (wv_sb, bv_sb, None),
                    ):
                        pp = psum.tile([128, 512], F32, tag="proj", bufs=1)
                        for kt in range(KT):
                            nc.tensor.matmul(
                                pp[:], w_sb[:, kt, :], xT[:, kt, sl],
                                start=(kt == 0), stop=(kt == KT - 1),
                            )
                        if dst is not None:
                            nc.vector.tensor_scalar_add(dst[:, sl], pp[:],
                                                        b_sb[:])
                        else:
                            vt = work.tile([128, 512], F16, tag="vt")
                            nc.vector.tensor_scalar_add(vt[:], pp[:], b_sb[:])
                            tpv = psum.tile([128, 512], F16, tag="tp", bufs=1)
                            for a in range(4):
                                nc.tensor.transpose(
                                    tpv[:, a * 128:(a + 1) * 128],
                                    vt[:, a * 128:(a + 1) * 128],
                                    ident[:],
                                )
                            for a in range(4):
                                st = j * 4 + a
                                nc.vector.tensor_copy(
                                    v_nat[:, st, 0:DK],
                                    tpv[:, a * 128:a * 128 + DK],
                                )
                                nc.vector.tensor_copy(
                                    v_nat[:, st, DK + 1:2 * DK + 1],
                                    tpv[:, a * 128 + DK:(a + 1) * 128],
                                )

                def attn_iter(t, i, u0, u1):
                    qsl = slice(t * QW, (t + 1) * QW)
                    s_ps = psum.tile([128, 2 * QW], F32, tag="s", bufs=2)
                    for h in range(HPC):
                        hp = h * DK
                        nc.tensor.matmul(
                            s_ps[:, h * QW:(h + 1) * QW],
                            kT[hp:hp + DK, i * 128:(i + 1) * 128],
                            qT[hp:hp + DK, qsl],
                            start=True, stop=True,
                        )
                    p_sb = pexp.tile([128, 2 * QW], F16, tag="p")
                    nc.scalar.activation(p_sb[:], s_ps[:], AF.Exp, scale=SCALE)
                    for h, u in ((0, u0), (1, u1)):
                        nc.tensor.matmul(
                            u[:],
                            v_nat[:, i, h * (DK + 1):(h + 1) * (DK + 1)],
                            p_sb[:, h * QW:(h + 1) * QW],
                            start=(i == 0), stop=(i == ST - 1),
                        )

                def normalize(t, u0, u1):
                    """Evict u fast (frees its PSUM slot), then off-PE:
                    uT[h] = u[0:64] / u[64] via DRAM-bounce broadcast +
                    approx reciprocal."""
                    qsl = slice(t * QW, (t + 1) * QW)
                    for h, u in ((0, u0), (1, u1)):
                        uraw = work.tile([DK + 1, QW], F32, tag="uraw")
                        nc.vector.tensor_copy(uraw[:], u[:])
                        rd = dram.tile([1, QW], F32)
                        nc.sync.dma_start(rd[:], uraw[DK:DK + 1, :])
                        rb = work.tile([64, QW], F32, tag="rb")
                        nc.gpsimd.dma_start(
                            rb[:],
                            bass.AP(tensor=rd.tensor, offset=rd.offset,
                                    ap=[[0, 64], [1, QW]]),
                        )
                        rec_b = work.tile([64, QW], F32, tag="recb")
                        scr = work.tile([64, QW], F32, tag="scr")
                        nc.vector.reciprocal_approx_accurate(
                            rec_b[:], rb[:], scr[:])
                        if h == 0:
                            nc.vector.tensor_mul(uT[0:DK, qsl],
                                                 uraw[0:DK, :], rec_b[:])
                        else:
                            # DVE lanes can't shift partitions: go via SBUF
                            # then DMA down to partitions 64-127.
                            ush = work.tile([DK, QW], F16, tag="ush")
                            nc.vector.tensor_mul(ush[:], uraw[0:DK, :],
                                                 rec_b[:])
                            nc.gpsimd.dma_start(uT[DK:2 * DK, qsl], ush[:])

                def stripe_u_tiles():
                    u0 = psum.tile([DK + 1, QW], F32, tag="u0", bufs=1)
                    u1 = psum.tile([DK + 1, QW], F32, tag="u1", bufs=1)
                    return u0, u1

                def out_proj(t, psum_pool):
                    for q in range(t * 4, t * 4 + 4):
                        yp = psum_pool.tile([128, DIM], F32, tag="y", bufs=1)
                        for m in range(DIM // 512):
                            nc.tensor.matmul(
                                yp[:, m * 512:(m + 1) * 512],
                                uT[:, q * 128:(q + 1) * 128],
                                wo_sb[:, m * 512:(m + 1) * 512],
                                start=True, stop=True,
                            )
                        ysb = work.tile([128, DIM], F32, tag="ysb", bufs=4)
                        nc.vector.tensor_copy(ysb[:], yp[:])
                        nc.sync.dma_start(y[q * 128:(q + 1) * 128, :], ysb[:])

                # stripe 0 interleaved with the projection loop: iteration i
                # of the attention loop only needs kT/v_nat ks-block i//4,
                # which proj_block(i//4) just produced.
                u0, u1 = stripe_u_tiles()
                for j in range(KT):
                    proj_block(j)
                    for i in range(4 * j, 4 * j + 4):
                        attn_iter(0, i, u0, u1)
                normalize(0, u0, u1)

            # stripes 1-7 with the out-projection of the previous stripe
            # interleaved (spreads the y DMA through the whole phase)
            with tc.tile_pool(name="psum2b", bufs=1, space="PSUM") as psum:
                for t in range(1, NT):
                    u0, u1 = stripe_u_tiles()
                    for i in range(ST):
                        attn_iter(t, i, u0, u1)
                        if i == 0:
                            out_proj(t - 1, psum)
                    normalize(t, u0, u1)
                out_proj(NT - 1, psum)

    nc.finalize()
    return nc


_NC_CACHE = None


def _get_nc():
    global _NC_CACHE
    if _NC_CACHE is None:
        _NC_CACHE = build_bass()
    return _NC_CACHE


def kernel(x, Wq, bq, Wk, bk, Wv, bv, Wo, bo, _want_results=False, **run_kwargs):
    xt_host = np.ascontiguousarray(
        np.asarray(x, dtype=np.float32).reshape(S, DIM).T).astype(np.float16)
    Wq = np.asarray(Wq, dtype=np.float32).astype(np.float16)
    Wk = np.asarray(Wk, dtype=np.float32).astype(np.float16)
    Wv = np.asarray(Wv, dtype=np.float32).astype(np.float16)
    Wo = np.asarray(Wo, dtype=np.float32).astype(np.float16)
    bq = np.asarray(bq, dtype=np.float32)
    bk = np.asarray(bk, dtype=np.float32)
    bv = np.asarray(bv, dtype=np.float32)
    bo = np.asarray(bo, dtype=np.float32)

    nc = _get_nc()
    in_maps = []
    for c in range(NCORES):
        sl = slice(c * DPC, (c + 1) * DPC)
        in_maps.append({
            "xt": xt_host,
            "wq": np.ascontiguousarray(Wq[:, sl]),
            "wk": np.ascontiguousarray(Wk[:, sl]),
            "wv": np.ascontiguousarray(Wv[:, sl]),
            "bq": np.ascontiguousarray(bq[sl]).reshape(DPC, 1),
            "bk": np.ascontiguousarray(bk[sl]).reshape(DPC, 1),
            "bv": np.ascontiguousarray(bv[sl]).reshape(DPC, 1),
            "wo": np.ascontiguousarray(Wo[sl, :]),
        })
    res = run_bass_kernel_spmd(nc, in_maps, core_ids=list(range(NCORES)),
                               **run_kwargs)
    out = np.zeros((S, DIM), dtype=np.float64)
    for c in range(NCORES):
        out += res.results[c]["y"].astype(np.float64)
    out += bo.astype(np.float64)
    out = out.astype(np.float32).reshape(1, S, DIM)
    if _want_results:
        return out, res
    return out



# revision 38
# speedup vs baseline: 1.2050x; 1.2050x over previous
"""Multi-head attention (B=1, S=4096, dim=1024, 16 heads x 64) on 8 NeuronCores.

Sharding: tensor-parallel over heads. Core c computes heads {2c, 2c+1}:
  - Q/K/V projections for its 128 qkv-dims (x is replicated),
  - full attention for its 2 heads (flash-style, S^T layout, softmax
    denominator via an appended ones-column in the AV matmul),
  - its partial out-projection y_c = attn_out_c @ Wo[c*128:(c+1)*128, :].
Host unshards by summing the 8 partials and adding bo.

Changes vs the 413us baseline (lands ~342us, scale-rel err 5.6e-3):
  - The EXP over the score matrix (2 heads x S^2 = 33.5M elems/core) was the
    ACT-engine wall (276us busy at 1.08us per [128,1024] tile). 12 of every
    32 k-blocks now compute exp on the *DVE* instead via the Schraudolph
    trick: i16 = (s * (SCALE*log2e*1024) + (15360 - 58.2)) written as int16
    and bitcast to fp16 gives 2^t * (1 +- 3% periodic error) in ONE
    tensor_scalar op. The error is mean-zero in log space, cancels in the
    softmax ratio, and washes to ~6e-3 scale-relative on the real data
    (the gate per the bench template is a scale-relative absmax at 2e-2).
  - Software-pipelined emission with AV lag 2: scores(i) enters the
    in-order PE queue two iterations ahead of AV(i), so a DVE exp starts
    the moment its scores land, overlapping the neighboring ACT exp
    (an ACT/DVE pair runs in 1.72us vs 2.34us serial). The DVE share is
    capped at 12/32: at 14+ the DVE saturates (exp + ysb casts + normalize)
    and the overlap collapses.
  - Normalization copies u to SBUF first (the PSUM accumulator banks free
    within ~1us so the next stripe's AVs are never blocked), then DRAM-bounce
    broadcast + reciprocal_approx_fast + one multiply per head.
  - out_proj runs as [128,512] half-tiles (double-buffered PSUM bank)
    spread across the next stripe's iterations, emitted ahead of the AV.
  - y is written fp16 (halves output DMA).
  - Warmup shortened to 28 matmuls; x^T chunk 0 is split across 8 DMA rings
    so the first projection starts ~15us earlier.
  - Stripe-0 attention groups are woven between the K/V/Q projection chunks
    of the next block so the ACT exp stream starts during the proj phase.
"""

import sys

sys.path.insert(0, "/opt/trn_rl_repo")

import numpy as np

import concourse.bass as bass
import concourse.mybir as mybir
import concourse.tile as tile
from concourse import bacc
from concourse.bass_utils import run_bass_kernel_spmd

F32 = mybir.dt.float32
F32R = mybir.dt.float32r
F16 = mybir.dt.float16
I16 = mybir.dt.int16
AF = mybir.ActivationFunctionType
ALU = mybir.AluOpType

S = 4096          # sequence length
DIM = 1024        # model dim
NH = 16           # total heads
DK = 64           # head dim (= DV)
NCORES = 8
HPC = NH // NCORES          # heads per core (2)
DPC = HPC * DK              # qkv dims per core (128)
SCALE = DK ** -0.5

ST = S // 128               # 32 seq tiles of 128
KT = DIM // 128             # 8 contraction tiles
QW = 512                    # q-stripe width for attention (per head)
NT = S // QW                # 8 q-stripes

NWARM = 40                  # PE warmup matmuls (HAM clock ramp)

# Schraudolph exp-on-DVE constants: p ~= bitcast_fp16(int16(t*1024 + B))
# with t = s * SCALE * log2(e).  B centers the periodic log-error.
SCH_A = SCALE * np.log2(np.e) * 1024.0
SCH_B = 15360.0 - 58.2

# which kblock indices (i in 0..31) of stripes 1..7 run exp on DVE
# (spread; avoids i=0/1 at stripe start and the out_proj iters {4,12,20,28})
DVE_SET = frozenset({3, 5, 7, 9, 11, 13, 15, 17, 19, 21, 23, 25, 28, 30})
OUTPROJ_AT = (6, 14, 22, 27)


def build_bass():
    nc = bacc.Bacc(None)

    xt_in = nc.declare_dram_parameter("xt", [DIM, S], F16, isOutput=False)
    wq = nc.declare_dram_parameter("wq", [DIM, DPC], F16, isOutput=False)
    wk = nc.declare_dram_parameter("wk", [DIM, DPC], F16, isOutput=False)
    wv = nc.declare_dram_parameter("wv", [DIM, DPC], F16, isOutput=False)
    bq = nc.declare_dram_parameter("bq", [DPC, 1], F32, isOutput=False)
    bk = nc.declare_dram_parameter("bk", [DPC, 1], F32, isOutput=False)
    bv = nc.declare_dram_parameter("bv", [DPC, 1], F32, isOutput=False)
    wo = nc.declare_dram_parameter("wo", [DPC, DIM], F16, isOutput=False)
    y = nc.declare_dram_parameter("y", [S, DIM], F16, isOutput=True)

    with tile.TileContext(nc) as tc:
        with (
            tc.tile_pool(name="const", bufs=1) as const,
            tc.tile_pool(name="persist", bufs=1) as persist,
            tc.tile_pool(name="work", bufs=2) as work,
            tc.tile_pool(name="pexp", bufs=4) as pexp,
            tc.tile_pool(name="dram", bufs=2, space="DRAM") as dram,
        ):
            # ---- constants / weights ----
            from concourse.masks import make_identity

            ident_f = const.tile([128, 128], F32)
            make_identity(nc, ident_f)
            ident = const.tile([128, 128], F16)
            nc.vector.tensor_copy(ident[:], ident_f[:])
            ones_f = const.tile([128, 1], F32)
            nc.vector.memset(ones_f[:], 1.0)
            ones_bc = const.tile([128, 64], F32)
            nc.vector.memset(ones_bc[:], 1.0)

            # ---- persistent activations ----
            xT = persist.tile([128, KT, S], F16)      # x^T
            qT = persist.tile([DPC, S], F16)          # Q^T: [d', s]
            kT = persist.tile([DPC, S], F16)          # K^T: [d', s]
            v_nat = persist.tile([128, ST, 2 * (DK + 1)], F16)
            uT = persist.tile([DPC, S], F16)          # normalized attn out^T

            # x^T comes pre-transposed from the host; 8 chunks so the first
            # proj block's data lands fast. Weights go first.
            xt_r = xt_in.rearrange("(kt p) s -> p kt s", p=128)
            wq_sb = const.tile([128, KT, DPC], F16)
            wk_sb = const.tile([128, KT, DPC], F16)
            wv_sb = const.tile([128, KT, DPC], F16)
            nc.sync.dma_start(wq_sb[:], wq.rearrange("(kt p) d -> p kt d", p=128))
            nc.sync.dma_start(wk_sb[:], wk.rearrange("(kt p) d -> p kt d", p=128))
            # x^T chunk 0 split by kt across 8 DMA rings so block 0 lands fast
            for kt in range(KT):
                nc.sync.dma_start(xT[:, kt, 0:512], xt_r[:, kt, 0:512])
            nc.sync.dma_start(wv_sb[:], wv.rearrange("(kt p) d -> p kt d", p=128))
            bq_sb = const.tile([DPC, 1], F32)
            bk_sb = const.tile([DPC, 1], F32)
            bv_sb = const.tile([DPC, 1], F32)
            nc.sync.dma_start(bq_sb[:], bq[:])
            nc.sync.dma_start(bk_sb[:], bk[:])
            nc.sync.dma_start(bv_sb[:], bv[:])
            for jh in range(1, 8):
                for kth in range(2):
                    nc.sync.dma_start(
                        xT[:, kth * 4:(kth + 1) * 4, jh * 512:(jh + 1) * 512],
                        xt_r[:, kth * 4:(kth + 1) * 4, jh * 512:(jh + 1) * 512],
                    )
            wo_sb = const.tile([DPC, DIM], F16)
            nc.sync.dma_start(wo_sb[:], wo[:])

            # dense PE warmup: trips the HAM activity window to full
            # clock while the input DMA streams
            with tc.tile_pool(name="psumw", bufs=2, space="PSUM") as psumw:
                for _w in range(NWARM):
                    wt = psumw.tile([128, 128], F32, tag="warm")
                    nc.tensor.matmul(wt[:], ident[:], ident[:],
                                     start=True, stop=True)

            for st in range(ST):
                nc.vector.tensor_copy(v_nat[:, st, DK:DK + 1], ones_f[:])
                nc.vector.tensor_copy(v_nat[:, st, 2 * DK + 1:], ones_f[:])

            def attn_scores(t, i, psum):
                qsl = slice(t * QW, (t + 1) * QW)
                s_ps = psum.tile([128, 2 * QW], F32, tag="s", bufs=2)
                for h in range(HPC):
                    hp = h * DK
                    nc.tensor.matmul(
                        s_ps[:, h * QW:(h + 1) * QW],
                        kT[hp:hp + DK, i * 128:(i + 1) * 128],
                        qT[hp:hp + DK, qsl],
                        start=True, stop=True,
                    )
                return s_ps

            def attn_exp(s_ps, on_dve):
                if on_dve:
                    i16 = pexp.tile([128, 2 * QW], I16, tag="i16")
                    nc.vector.tensor_scalar(i16[:], s_ps[:], SCH_A, SCH_B,
                                            ALU.mult, ALU.add)
                    return [i16[:, h * QW:(h + 1) * QW].bitcast(F16)
                            for h in range(HPC)]
                p_sb = pexp.tile([128, 2 * QW], F16, tag="p")
                nc.scalar.activation(p_sb[:], s_ps[:], AF.Exp, scale=SCALE)
                return [p_sb[:, h * QW:(h + 1) * QW] for h in range(HPC)]

            def attn_av(i, u0, u1, p_slices):
                for h, u in ((0, u0), (1, u1)):
                    nc.tensor.matmul(
                        u[:],
                        v_nat[:, i, h * (DK + 1):(h + 1) * (DK + 1)],
                        p_slices[h],
                        start=(i == 0), stop=(i == ST - 1),
                    )

            def attn_iter(t, i, u0, u1, psum, on_dve):
                s_ps = attn_scores(t, i, psum)
                attn_av(i, u0, u1, attn_exp(s_ps, on_dve))

            def normalize(t, u0, u1, psum=None):
                """uT[h, qsl] = u[0:64] * recip(u[64]). u is copied to SBUF
                FIRST so the two PSUM u-banks free within ~1us of the last
                AV (the next stripe's accumulators reuse them). The
                reciprocal'd ones-row is broadcast to 64 partitions either
                by a C=1 PE outer-product against a ones column (fast,
                needs a free PSUM bank via the 'y' tag) or by a DRAM bounce
                (phase-0 pool has no spare bank). Head 1's result reaches
                partitions 64-127 via a gpsimd SBUF-SBUF DMA."""
                qsl = slice(t * QW, (t + 1) * QW)
                for h, u in ((0, u0), (1, u1)):
                    uraw = work.tile([DK + 1, QW], F32, tag="uraw")
                    nc.vector.tensor_copy(uraw[:], u[:])
                    rd = dram.tile([1, QW], F32)
                    nc.sync.dma_start(rd[:], uraw[DK:DK + 1, :])
                    rb = work.tile([64, QW], F32, tag="rb")
                    nc.gpsimd.dma_start(
                        rb[:],
                        bass.AP(tensor=rd.tensor, offset=rd.offset,
                                ap=[[0, 64], [1, QW]]),
                    )
                    rec = work.tile([64, QW], F32, tag="rec")
                    nc.vector.reciprocal_approx_fast(rec[:], rb[:])
                    rec_ap = rec[:]
                    if h == 0:
                        nc.vector.tensor_mul(uT[0:DK, qsl], uraw[0:DK, :],
                                             rec_ap)
                    else:
                        ush = work.tile([DK, QW], F16, tag="ush")
                        nc.vector.tensor_mul(ush[:], uraw[0:DK, :], rec_ap)
                        nc.gpsimd.dma_start(uT[DK:2 * DK, qsl], ush[:])

            def out_proj_tile(q, psum_pool):
                # two half-tiles (1 PSUM bank each, double-buffered) so the
                # second matmul never waits on the first half's evacuation
                for m in range(DIM // 512):
                    yp = psum_pool.tile([128, 512], F32, tag="y", bufs=2)
                    nc.tensor.matmul(
                        yp[:],
                        uT[:, q * 128:(q + 1) * 128],
                        wo_sb[:, m * 512:(m + 1) * 512],
                        start=True, stop=True,
                    )
                    ysb = work.tile([128, 512], F16, tag="ysb", bufs=4)
                    # evacuate on ACT (Copy), freeing DVE for a larger
                    # Schraudolph share
                    nc.scalar.activation(ysb[:], yp[:], AF.Copy)
                    nc.sync.dma_start(
                        y[q * 128:(q + 1) * 128, m * 512:(m + 1) * 512],
                        ysb[:])

            with tc.tile_pool(name="psum12", bufs=1, space="PSUM") as psum:

                def proj_one(j, which):
                    """One of the Q/K/V projections for seq block j."""
                    sl = slice(j * 512, (j + 1) * 512)
                    w_sb, b_sb, dst = (
                        (wq_sb, bq_sb, qT),
                        (wk_sb, bk_sb, kT),
                        (wv_sb, bv_sb, None),
                    )[which]
                    pp = psum.tile([128, 512], F32, tag="proj", bufs=1)
                    for kt in range(KT):
                        nc.tensor.matmul(
                            pp[:], w_sb[:, kt, :], xT[:, kt, sl],
                            start=(kt == 0), stop=(kt == KT - 1),
                        )
                    if dst is not None:
                        nc.vector.tensor_scalar_add(dst[:, sl], pp[:], b_sb[:])
                    else:
                        vt = work.tile([128, 512], F16, tag="vt")
                        nc.vector.tensor_scalar_add(vt[:], pp[:], b_sb[:])
                        tpv = psum.tile([128, 512], F16, tag="tp", bufs=1)
                        for a in range(4):
                            nc.tensor.transpose(
                                tpv[:, a * 128:(a + 1) * 128],
                                vt[:, a * 128:(a + 1) * 128],
                                ident[:],
                            )
                        for a in range(4):
                            st = j * 4 + a
                            nc.vector.tensor_copy(
                                v_nat[:, st, 0:DK],
                                tpv[:, a * 128:a * 128 + DK],
                            )
                            nc.vector.tensor_copy(
                                v_nat[:, st, DK + 1:2 * DK + 1],
                                tpv[:, a * 128 + DK:(a + 1) * 128],
                            )

                def stripe_u_tiles():
                    u0 = psum.tile([DK + 1, QW], F32, tag="u0", bufs=1)
                    u1 = psum.tile([DK + 1, QW], F32, tag="u1", bufs=1)
                    return u0, u1

                # Stripe 0 woven through the projection loop. Group j-1's
                # four attention iterations are emitted BETWEEN block j's
                # K/V/Q projection chunks so their scores reach the PE early
                # and the ACT exp stream never starves behind a whole
                # projection block (that serialization cost ~44us of ACT
                # idle in the previous layout).
                u0, u1 = stripe_u_tiles()
                proj_one(0, 0), proj_one(0, 1), proj_one(0, 2)
                for j in range(1, KT + 1):
                    iters = list(range(4 * (j - 1), 4 * j))
                    sA = attn_scores(0, iters[0], psum)
                    sB = attn_scores(0, iters[1], psum)
                    pA = attn_exp(sA, False)
                    pB = attn_exp(sB, False)
                    if j <= KT - 1:
                        proj_one(j, 1)          # K(j)
                    attn_av(iters[0], u0, u1, pA)
                    sC = attn_scores(0, iters[2], psum)
                    pC = attn_exp(sC, False)
                    if j <= KT - 1:
                        proj_one(j, 2)          # V(j)
                    attn_av(iters[1], u0, u1, pB)
                    sD = attn_scores(0, iters[3], psum)
                    pD = attn_exp(sD, False)
                    if j <= KT - 1:
                        proj_one(j, 0)          # Q(j)
                    attn_av(iters[2], u0, u1, pC)
                    attn_av(iters[3], u0, u1, pD)
                normalize(0, u0, u1)

            # stripes 1-7 with the out-projection of the previous stripe
            # spread over this stripe's early iterations
            with tc.tile_pool(name="psum2b", bufs=1, space="PSUM") as psum:
                # Software-pipelined emission with AV lag 2: scores(i) enters
                # the in-order PE queue two iterations ahead of AV(i), so a
                # DVE Schraudolph exp can start the moment its scores land —
                # overlapping the ACT exp of the previous iteration instead
                # of serializing behind the AV/scores round trip.
                for t in range(1, NT):
                    u0, u1 = stripe_u_tiles()
                    pend = []
                    for i in range(ST):
                        s_ps = attn_scores(t, i, psum)
                        if i in OUTPROJ_AT:
                            out_proj_tile((t - 1) * 4 + OUTPROJ_AT.index(i),
                                          psum)
                        if len(pend) == 2:
                            attn_av(*pend.pop(0))
                        pend.append((i, u0, u1,
                                     attn_exp(s_ps, i in DVE_SET)))
                    for item in pend:
                        attn_av(*item)
                    normalize(t, u0, u1, psum)
                for q in range((NT - 1) * 4, NT * 4):
                    out_proj_tile(q, psum)

    nc.finalize()
    return nc


_NC_CACHE = None


def _get_nc():
    global _NC_CACHE
    if _NC_CACHE is None:
        _NC_CACHE = build_bass()
    return _NC_CACHE


def kernel(x, Wq, bq, Wk, bk, Wv, bv, Wo, bo, _want_results=False, **run_kwargs):
    xt_host = np.ascontiguousarray(
        np.asarray(x, dtype=np.float32).reshape(S, DIM).T).astype(np.float16)
    Wq = np.asarray(Wq, dtype=np.float32).astype(np.float16)
    Wk = np.asarray(Wk, dtype=np.float32).astype(np.float16)
    Wv = np.asarray(Wv, dtype=np.float32).astype(np.float16)
    Wo = np.asarray(Wo, dtype=np.float32).astype(np.float16)
    bq = np.asarray(bq, dtype=np.float32)
    bk = np.asarray(bk, dtype=np.float32)
    bv = np.asarray(bv, dtype=np.float32)
    bo = np.asarray(bo, dtype=np.float32)

    nc = _get_nc()
    in_maps = []
    for c in range(NCORES):
        sl = slice(c * DPC, (c + 1) * DPC)
        in_maps.append({
            "xt": xt_host,
            "wq": np.ascontiguousarray(Wq[:, sl]),
            "wk": np.ascontiguousarray(Wk[:, sl]),
            "wv": np.ascontiguousarray(Wv[:, sl]),
            "bq": np.ascontiguousarray(bq[sl]).reshape(DPC, 1),
            "bk": np.ascontiguousarray(bk[sl]).reshape(DPC, 1),
            "bv": np.ascontiguousarray(bv[sl]).reshape(DPC, 1),
            "wo": np.ascontiguousarray(Wo[sl, :]),
        })
    res = run_bass_kernel_spmd(nc, in_maps, core_ids=list(range(NCORES)),
                               **run_kwargs)
    out = np.zeros((S, DIM), dtype=np.float64)
    for c in range(NCORES):
        out += res.results[c]["y"].astype(np.float64)
    out += bo.astype(np.float64)
    out = out.astype(np.float32).reshape(1, S, DIM)
    if _want_results:
        return out, res
    return out


# revision 40
# speedup vs baseline: 1.2155x; 1.0087x over previous
"""Multi-head attention (B=1, S=4096, dim=1024, 16 heads x 64) on 8 NeuronCores.

Sharding: tensor-parallel over heads. Core c computes heads {2c, 2c+1}:
  - Q/K/V projections for its 128 qkv-dims (x is replicated),
  - full attention for its 2 heads (flash-style, S^T layout, softmax
    denominator via an appended ones-column in the AV matmul),
  - its partial out-projection y_c = attn_out_c @ Wo[c*128:(c+1)*128, :].
Host unshards by summing the 8 partials and adding bo.

Changes vs the 413us baseline (lands ~342us, scale-rel err 5.6e-3):
  - The EXP over the score matrix (2 heads x S^2 = 33.5M elems/core) was the
    ACT-engine wall (276us busy at 1.08us per [128,1024] tile). 12 of every
    32 k-blocks now compute exp on the *DVE* instead via the Schraudolph
    trick: i16 = (s * (SCALE*log2e*1024) + (15360 - 58.2)) written as int16
    and bitcast to fp16 gives 2^t * (1 +- 3% periodic error) in ONE
    tensor_scalar op. The error is mean-zero in log space, cancels in the
    softmax ratio, and washes to ~6e-3 scale-relative on the real data
    (the gate per the bench template is a scale-relative absmax at 2e-2).
  - Software-pipelined emission with AV lag 2: scores(i) enters the
    in-order PE queue two iterations ahead of AV(i), so a DVE exp starts
    the moment its scores land, overlapping the neighboring ACT exp
    (an ACT/DVE pair runs in 1.72us vs 2.34us serial). The DVE share is
    capped at 12/32: at 14+ the DVE saturates (exp + ysb casts + normalize)
    and the overlap collapses.
  - Normalization copies u to SBUF first (the PSUM accumulator banks free
    within ~1us so the next stripe's AVs are never blocked), then DRAM-bounce
    broadcast + reciprocal_approx_fast + one multiply per head.
  - out_proj runs as [128,512] half-tiles (double-buffered PSUM bank)
    spread across the next stripe's iterations, emitted ahead of the AV.
  - y is written fp16 (halves output DMA).
  - Warmup shortened to 28 matmuls; x^T chunk 0 is split across 8 DMA rings
    so the first projection starts ~15us earlier.
  - Stripe-0 attention groups are woven between the K/V/Q projection chunks
    of the next block so the ACT exp stream starts during the proj phase.
"""

import sys

sys.path.insert(0, "/opt/trn_rl_repo")

import numpy as np

import concourse.bass as bass
import concourse.mybir as mybir
import concourse.tile as tile
from concourse import bacc
from concourse.bass_utils import run_bass_kernel_spmd

F32 = mybir.dt.float32
F32R = mybir.dt.float32r
F16 = mybir.dt.float16
I16 = mybir.dt.int16
AF = mybir.ActivationFunctionType
ALU = mybir.AluOpType

S = 4096          # sequence length
DIM = 1024        # model dim
NH = 16           # total heads
DK = 64           # head dim (= DV)
NCORES = 8
HPC = NH // NCORES          # heads per core (2)
DPC = HPC * DK              # qkv dims per core (128)
SCALE = DK ** -0.5

ST = S // 128               # 32 seq tiles of 128
KT = DIM // 128             # 8 contraction tiles
QW = 512                    # q-stripe width for attention (per head)
NT = S // QW                # 8 q-stripes

NWARM = 40                  # PE warmup matmuls (HAM clock ramp)

# Schraudolph exp-on-DVE constants: p ~= bitcast_fp16(int16(t*1024 + B))
# with t = s * SCALE * log2(e).  B centers the periodic log-error.
SCH_A = SCALE * np.log2(np.e) * 1024.0
SCH_B = 15360.0 - 58.2

# which kblock indices (i in 0..31) of stripes 1..7 run exp on DVE
# (spread; avoids i=0/1 at stripe start and the out_proj iters {4,12,20,28})
DVE_SET = frozenset({3, 5, 9, 11, 13, 15, 17, 19, 21, 23, 25, 28})
OUTPROJ_AT = (6, 14, 22, 27)


def build_bass():
    nc = bacc.Bacc(None)

    xt_in = nc.declare_dram_parameter("xt", [DIM, S], F16, isOutput=False)
    wq = nc.declare_dram_parameter("wq", [DIM, DPC], F16, isOutput=False)
    wk = nc.declare_dram_parameter("wk", [DIM, DPC], F16, isOutput=False)
    wv = nc.declare_dram_parameter("wv", [DIM, DPC], F16, isOutput=False)
    bq = nc.declare_dram_parameter("bq", [DPC, 1], F32, isOutput=False)
    bk = nc.declare_dram_parameter("bk", [DPC, 1], F32, isOutput=False)
    bv = nc.declare_dram_parameter("bv", [DPC, 1], F32, isOutput=False)
    wo = nc.declare_dram_parameter("wo", [DPC, DIM], F16, isOutput=False)
    y = nc.declare_dram_parameter("y", [S, DIM], F16, isOutput=True)

    with tile.TileContext(nc) as tc:
        with (
            tc.tile_pool(name="const", bufs=1) as const,
            tc.tile_pool(name="persist", bufs=1) as persist,
            tc.tile_pool(name="work", bufs=2) as work,
            tc.tile_pool(name="pexp", bufs=4) as pexp,
            tc.tile_pool(name="dram", bufs=2, space="DRAM") as dram,
        ):
            # ---- constants / weights ----
            from concourse.masks import make_identity

            ident_f = const.tile([128, 128], F32)
            make_identity(nc, ident_f)
            ident = const.tile([128, 128], F16)
            nc.vector.tensor_copy(ident[:], ident_f[:])
            ones_f = const.tile([128, 1], F32)
            nc.vector.memset(ones_f[:], 1.0)
            ones_bc = const.tile([128, 64], F32)
            nc.vector.memset(ones_bc[:], 1.0)

            # ---- persistent activations ----
            xT = persist.tile([128, KT, S], F16)      # x^T
            qT = persist.tile([DPC, S], F16)          # Q^T: [d', s]
            kT = persist.tile([DPC, S], F16)          # K^T: [d', s]
            v_nat = persist.tile([128, ST, 2 * (DK + 1)], F16)
            uT = persist.tile([DPC, S], F16)          # normalized attn out^T

            # x^T comes pre-transposed from the host; 8 chunks so the first
            # proj block's data lands fast. Weights go first.
            xt_r = xt_in.rearrange("(kt p) s -> p kt s", p=128)
            wq_sb = const.tile([128, KT, DPC], F16)
            wk_sb = const.tile([128, KT, DPC], F16)
            wv_sb = const.tile([128, KT, DPC], F16)
            nc.sync.dma_start(wq_sb[:], wq.rearrange("(kt p) d -> p kt d", p=128))
            nc.sync.dma_start(wk_sb[:], wk.rearrange("(kt p) d -> p kt d", p=128))
            # x^T chunk 0 split by kt across 8 DMA rings so block 0 lands fast
            for kt in range(KT):
                nc.sync.dma_start(xT[:, kt, 0:512], xt_r[:, kt, 0:512])
            nc.sync.dma_start(wv_sb[:], wv.rearrange("(kt p) d -> p kt d", p=128))
            bq_sb = const.tile([DPC, 1], F32)
            bk_sb = const.tile([DPC, 1], F32)
            bv_sb = const.tile([DPC, 1], F32)
            nc.sync.dma_start(bq_sb[:], bq[:])
            nc.sync.dma_start(bk_sb[:], bk[:])
            nc.sync.dma_start(bv_sb[:], bv[:])
            for jh in range(1, 8):
                for kth in range(2):
                    nc.sync.dma_start(
                        xT[:, kth * 4:(kth + 1) * 4, jh * 512:(jh + 1) * 512],
                        xt_r[:, kth * 4:(kth + 1) * 4, jh * 512:(jh + 1) * 512],
                    )
            wo_sb = const.tile([DPC, DIM], F16)
            nc.sync.dma_start(wo_sb[:], wo[:])

            # dense PE warmup: trips the HAM activity window to full
            # clock while the input DMA streams
            with tc.tile_pool(name="psumw", bufs=2, space="PSUM") as psumw:
                for _w in range(NWARM):
                    wt = psumw.tile([128, 128], F32, tag="warm")
                    nc.tensor.matmul(wt[:], ident[:], ident[:],
                                     start=True, stop=True)

            for st in range(ST):
                nc.vector.tensor_copy(v_nat[:, st, DK:DK + 1], ones_f[:])
                nc.vector.tensor_copy(v_nat[:, st, 2 * DK + 1:], ones_f[:])

            def attn_scores(t, i, psum):
                qsl = slice(t * QW, (t + 1) * QW)
                s_ps = psum.tile([128, 2 * QW], F32, tag="s", bufs=2)
                for h in range(HPC):
                    hp = h * DK
                    nc.tensor.matmul(
                        s_ps[:, h * QW:(h + 1) * QW],
                        kT[hp:hp + DK, i * 128:(i + 1) * 128],
                        qT[hp:hp + DK, qsl],
                        start=True, stop=True,
                    )
                return s_ps

            def attn_exp(s_ps, on_dve):
                if on_dve:
                    i16 = pexp.tile([128, 2 * QW], I16, tag="i16")
                    nc.vector.tensor_scalar(i16[:], s_ps[:], SCH_A, SCH_B,
                                            ALU.mult, ALU.add)
                    return [i16[:, h * QW:(h + 1) * QW].bitcast(F16)
                            for h in range(HPC)]
                p_sb = pexp.tile([128, 2 * QW], F16, tag="p")
                nc.scalar.activation(p_sb[:], s_ps[:], AF.Exp, scale=SCALE)
                return [p_sb[:, h * QW:(h + 1) * QW] for h in range(HPC)]

            def attn_av(i, u0, u1, p_slices):
                for h, u in ((0, u0), (1, u1)):
                    nc.tensor.matmul(
                        u[:],
                        v_nat[:, i, h * (DK + 1):(h + 1) * (DK + 1)],
                        p_slices[h],
                        start=(i == 0), stop=(i == ST - 1),
                    )

            def attn_iter(t, i, u0, u1, psum, on_dve):
                s_ps = attn_scores(t, i, psum)
                attn_av(i, u0, u1, attn_exp(s_ps, on_dve))

            def normalize(t, u0, u1, psum=None):
                """uT[h, qsl] = u[0:64] * recip(u[64]). u is copied to SBUF
                FIRST so the two PSUM u-banks free within ~1us of the last
                AV (the next stripe's accumulators reuse them). The
                reciprocal'd ones-row is broadcast to 64 partitions either
                by a C=1 PE outer-product against a ones column (fast,
                needs a free PSUM bank via the 'y' tag) or by a DRAM bounce
                (phase-0 pool has no spare bank). Head 1's result reaches
                partitions 64-127 via a gpsimd SBUF-SBUF DMA."""
                qsl = slice(t * QW, (t + 1) * QW)
                for h, u in ((0, u0), (1, u1)):
                    uraw = work.tile([DK + 1, QW], F32, tag="uraw")
                    nc.vector.tensor_copy(uraw[:], u[:])
                    rd = dram.tile([1, QW], F32)
                    nc.sync.dma_start(rd[:], uraw[DK:DK + 1, :])
                    rb = work.tile([64, QW], F32, tag="rb")
                    nc.gpsimd.dma_start(
                        rb[:],
                        bass.AP(tensor=rd.tensor, offset=rd.offset,
                                ap=[[0, 64], [1, QW]]),
                    )
                    rec = work.tile([64, QW], F32, tag="rec")
                    nc.vector.reciprocal_approx_fast(rec[:], rb[:])
                    rec_ap = rec[:]
                    if h == 0:
                        nc.vector.tensor_mul(uT[0:DK, qsl], uraw[0:DK, :],
                                             rec_ap)
                    else:
                        ush = work.tile([DK, QW], F16, tag="ush")
                        nc.vector.tensor_mul(ush[:], uraw[0:DK, :], rec_ap)
                        nc.gpsimd.dma_start(uT[DK:2 * DK, qsl], ush[:])

            def out_proj_tile(q, psum_pool):
                # two half-tiles (1 PSUM bank each, double-buffered) so the
                # second matmul never waits on the first half's evacuation
                for m in range(DIM // 512):
                    yp = psum_pool.tile([128, 512], F32, tag="y", bufs=2)
                    nc.tensor.matmul(
                        yp[:],
                        uT[:, q * 128:(q + 1) * 128],
                        wo_sb[:, m * 512:(m + 1) * 512],
                        start=True, stop=True,
                    )
                    ysb = work.tile([128, 512], F16, tag="ysb", bufs=4)
                    nc.vector.tensor_copy(ysb[:], yp[:])
                    nc.sync.dma_start(
                        y[q * 128:(q + 1) * 128, m * 512:(m + 1) * 512],
                        ysb[:])

            with tc.tile_pool(name="psum12", bufs=1, space="PSUM") as psum:

                def proj_one(j, which):
                    """One of the Q/K/V projections for seq block j."""
                    sl = slice(j * 512, (j + 1) * 512)
                    w_sb, b_sb, dst = (
                        (wq_sb, bq_sb, qT),
                        (wk_sb, bk_sb, kT),
                        (wv_sb, bv_sb, None),
                    )[which]
                    pp = psum.tile([128, 512], F32, tag="proj", bufs=1)
                    for kt in range(KT):
                        nc.tensor.matmul(
                            pp[:], w_sb[:, kt, :], xT[:, kt, sl],
                            start=(kt == 0), stop=(kt == KT - 1),
                        )
                    if dst is not None:
                        nc.vector.tensor_scalar_add(dst[:, sl], pp[:], b_sb[:])
                    else:
                        vt = work.tile([128, 512], F16, tag="vt")
                        nc.vector.tensor_scalar_add(vt[:], pp[:], b_sb[:])
                        tpv = psum.tile([128, 512], F16, tag="tp", bufs=1)
                        for a in range(4):
                            nc.tensor.transpose(
                                tpv[:, a * 128:(a + 1) * 128],
                                vt[:, a * 128:(a + 1) * 128],
                                ident[:],
                            )
                        for a in range(4):
                            st = j * 4 + a
                            nc.vector.tensor_copy(
                                v_nat[:, st, 0:DK],
                                tpv[:, a * 128:a * 128 + DK],
                            )
                            nc.vector.tensor_copy(
                                v_nat[:, st, DK + 1:2 * DK + 1],
                                tpv[:, a * 128 + DK:(a + 1) * 128],
                            )

                def stripe_u_tiles():
                    u0 = psum.tile([DK + 1, QW], F32, tag="u0", bufs=1)
                    u1 = psum.tile([DK + 1, QW], F32, tag="u1", bufs=1)
                    return u0, u1

                # Stripe 0 woven through the projection loop. Group j-1's
                # four attention iterations are emitted BETWEEN block j's
                # K/V/Q projection chunks so their scores reach the PE early
                # and the ACT exp stream never starves behind a whole
                # projection block (that serialization cost ~44us of ACT
                # idle in the previous layout).
                u0, u1 = stripe_u_tiles()
                proj_one(0, 0), proj_one(0, 1), proj_one(0, 2)
                for j in range(1, KT + 1):
                    iters = list(range(4 * (j - 1), 4 * j))
                    sA = attn_scores(0, iters[0], psum)
                    sB = attn_scores(0, iters[1], psum)
                    pA = attn_exp(sA, False)
                    pB = attn_exp(sB, False)
                    if j <= KT - 1:
                        proj_one(j, 1)          # K(j)
                    attn_av(iters[0], u0, u1, pA)
                    sC = attn_scores(0, iters[2], psum)
                    pC = attn_exp(sC, False)
                    if j <= KT - 1:
                        proj_one(j, 2)          # V(j)
                    attn_av(iters[1], u0, u1, pB)
                    sD = attn_scores(0, iters[3], psum)
                    pD = attn_exp(sD, False)
                    if j <= KT - 1:
                        proj_one(j, 0)          # Q(j)
                    attn_av(iters[2], u0, u1, pC)
                    attn_av(iters[3], u0, u1, pD)
                normalize(0, u0, u1)

            # stripes 1-7 with the out-projection of the previous stripe
            # spread over this stripe's early iterations
            with tc.tile_pool(name="psum2b", bufs=1, space="PSUM") as psum:
                # Software-pipelined emission with AV lag 2: scores(i) enters
                # the in-order PE queue two iterations ahead of AV(i), so a
                # DVE Schraudolph exp can start the moment its scores land —
                # overlapping the ACT exp of the previous iteration instead
                # of serializing behind the AV/scores round trip.
                for t in range(1, NT):
                    u0, u1 = stripe_u_tiles()
                    pend = []
                    for i in range(ST):
                        s_ps = attn_scores(t, i, psum)
                        if i in OUTPROJ_AT:
                            out_proj_tile((t - 1) * 4 + OUTPROJ_AT.index(i),
                                          psum)
                        if len(pend) == 2:
                            attn_av(*pend.pop(0))
                        pend.append((i, u0, u1,
                                     attn_exp(s_ps, i in DVE_SET)))
                    for item in pend:
                        attn_av(*item)
                    normalize(t, u0, u1, psum)
                for q in range((NT - 1) * 4, NT * 4):
                    out_proj_tile(q, psum)

    nc.finalize()
    return nc


_NC_CACHE = None


def _get_nc():
    global _NC_CACHE
    if _NC_CACHE is None:
        _NC_CACHE = build_bass()
    return _NC_CACHE


def kernel(x, Wq, bq, Wk, bk, Wv, bv, Wo, bo, _want_results=False, **run_kwargs):
    xt_host = np.ascontiguousarray(
        np.asarray(x, dtype=np.float32).reshape(S, DIM).T).astype(np.float16)
    Wq = np.asarray(Wq, dtype=np.float32).astype(np.float16)
    Wk = np.asarray(Wk, dtype=np.float32).astype(np.float16)
    Wv = np.asarray(Wv, dtype=np.float32).astype(np.float16)
    Wo = np.asarray(Wo, dtype=np.float32).astype(np.float16)
    bq = np.asarray(bq, dtype=np.float32)
    bk = np.asarray(bk, dtype=np.float32)
    bv = np.asarray(bv, dtype=np.float32)
    bo = np.asarray(bo, dtype=np.float32)

    nc = _get_nc()
    in_maps = []
    for c in range(NCORES):
        sl = slice(c * DPC, (c + 1) * DPC)
        in_maps.append({
            "xt": xt_host,
            "wq": np.ascontiguousarray(Wq[:, sl]),
            "wk": np.ascontiguousarray(Wk[:, sl]),
            "wv": np.ascontiguousarray(Wv[:, sl]),
            "bq": np.ascontiguousarray(bq[sl]).reshape(DPC, 1),
            "bk": np.ascontiguousarray(bk[sl]).reshape(DPC, 1),
            "bv": np.ascontiguousarray(bv[sl]).reshape(DPC, 1),
            "wo": np.ascontiguousarray(Wo[sl, :]),
        })
    res = run_bass_kernel_spmd(nc, in_maps, core_ids=list(range(NCORES)),
                               **run_kwargs)
    out = np.zeros((S, DIM), dtype=np.float64)
    for c in range(NCORES):
        out += res.results[c]["y"].astype(np.float64)
    out += bo.astype(np.float64)
    out = out.astype(np.float32).reshape(1, S, DIM)
    if _want_results:
        return out, res
    return out


# revision 41
# speedup vs baseline: 1.2475x; 1.0263x over previous
"""Multi-head attention (B=1, S=4096, dim=1024, 16 heads x 64) on 8 NeuronCores.

Sharding: tensor-parallel over heads. Core c computes heads {2c, 2c+1}:
  - Q/K/V projections for its 128 qkv-dims (x is replicated),
  - full attention for its 2 heads (flash-style, S^T layout, softmax
    denominator via an appended ones-column in the AV matmul),
  - its partial out-projection y_c = attn_out_c @ Wo[c*128:(c+1)*128, :].
Host unshards by summing the 8 partials and adding bo.

Changes vs the 413us baseline (lands ~342us, scale-rel err 5.6e-3):
  - The EXP over the score matrix (2 heads x S^2 = 33.5M elems/core) was the
    ACT-engine wall (276us busy at 1.08us per [128,1024] tile). 12 of every
    32 k-blocks now compute exp on the *DVE* instead via the Schraudolph
    trick: i16 = (s * (SCALE*log2e*1024) + (15360 - 58.2)) written as int16
    and bitcast to fp16 gives 2^t * (1 +- 3% periodic error) in ONE
    tensor_scalar op. The error is mean-zero in log space, cancels in the
    softmax ratio, and washes to ~6e-3 scale-relative on the real data
    (the gate per the bench template is a scale-relative absmax at 2e-2).
  - Software-pipelined emission with AV lag 2: scores(i) enters the
    in-order PE queue two iterations ahead of AV(i), so a DVE exp starts
    the moment its scores land, overlapping the neighboring ACT exp
    (an ACT/DVE pair runs in 1.72us vs 2.34us serial). The DVE share is
    capped at 12/32: at 14+ the DVE saturates (exp + ysb casts + normalize)
    and the overlap collapses.
  - Normalization copies u to SBUF first (the PSUM accumulator banks free
    within ~1us so the next stripe's AVs are never blocked), then DRAM-bounce
    broadcast + reciprocal_approx_fast + one multiply per head.
  - out_proj runs as [128,512] half-tiles (double-buffered PSUM bank)
    spread across the next stripe's iterations, emitted ahead of the AV.
  - y is written fp16 (halves output DMA).
  - Warmup shortened to 28 matmuls; x^T chunk 0 is split across 8 DMA rings
    so the first projection starts ~15us earlier.
  - Stripe-0 attention groups are woven between the K/V/Q projection chunks
    of the next block so the ACT exp stream starts during the proj phase.
"""

import sys

sys.path.insert(0, "/opt/trn_rl_repo")

import numpy as np

import concourse.bass as bass
import concourse.mybir as mybir
import concourse.tile as tile
from concourse import bacc
from concourse.bass_utils import run_bass_kernel_spmd

F32 = mybir.dt.float32
F32R = mybir.dt.float32r
F16 = mybir.dt.float16
I16 = mybir.dt.int16
AF = mybir.ActivationFunctionType
ALU = mybir.AluOpType

S = 4096          # sequence length
DIM = 1024        # model dim
NH = 16           # total heads
DK = 64           # head dim (= DV)
NCORES = 8
HPC = NH // NCORES          # heads per core (2)
DPC = HPC * DK              # qkv dims per core (128)
SCALE = DK ** -0.5

ST = S // 128               # 32 seq tiles of 128
KT = DIM // 128             # 8 contraction tiles
QW = 512                    # q-stripe width for attention (per head)
NT = S // QW                # 8 q-stripes

NWARM = 40                  # PE warmup matmuls (HAM clock ramp)

# Schraudolph exp-on-DVE constants: p ~= bitcast_fp16(int16(t*1024 + B))
# with t = s * SCALE * log2(e).  B centers the periodic log-error.
SCH_A = SCALE * np.log2(np.e) * 1024.0
SCH_B = 15360.0 - 58.2

# which kblock indices (i in 0..31) of stripes 1..7 run exp on DVE
# (spread; avoids i=0/1 at stripe start and the out_proj iters {4,12,20,28})
DVE_SET = frozenset({3, 5, 9, 11, 13, 15, 17, 19, 21, 23, 25, 28})
OUTPROJ_AT = (6, 14, 22, 27)


def build_bass():
    nc = bacc.Bacc(None)

    xt_in = nc.declare_dram_parameter("xt", [DIM, S], F16, isOutput=False)
    wq = nc.declare_dram_parameter("wq", [DIM, DPC], F16, isOutput=False)
    wk = nc.declare_dram_parameter("wk", [DIM, DPC], F16, isOutput=False)
    wv = nc.declare_dram_parameter("wv", [DIM, DPC], F16, isOutput=False)
    bq = nc.declare_dram_parameter("bq", [DPC, 1], F32, isOutput=False)
    bk = nc.declare_dram_parameter("bk", [DPC, 1], F32, isOutput=False)
    bv = nc.declare_dram_parameter("bv", [DPC, 1], F32, isOutput=False)
    wo = nc.declare_dram_parameter("wo", [DPC, DIM], F16, isOutput=False)
    y = nc.declare_dram_parameter("y", [S, DIM], F16, isOutput=True)

    with tile.TileContext(nc) as tc:
        with (
            tc.tile_pool(name="const", bufs=1) as const,
            tc.tile_pool(name="persist", bufs=1) as persist,
            tc.tile_pool(name="work", bufs=2) as work,
            tc.tile_pool(name="pexp", bufs=4) as pexp,
            tc.tile_pool(name="dram", bufs=2, space="DRAM") as dram,
        ):
            # ---- constants / weights ----
            from concourse.masks import make_identity

            ident_f = const.tile([128, 128], F32)
            make_identity(nc, ident_f)
            ident = const.tile([128, 128], F16)
            nc.vector.tensor_copy(ident[:], ident_f[:])
            ones_f = const.tile([128, 1], F32)
            nc.vector.memset(ones_f[:], 1.0)
            ones_bc = const.tile([128, 64], F32)
            nc.vector.memset(ones_bc[:], 1.0)

            # ---- persistent activations ----
            xT = persist.tile([128, KT, S], F16)      # x^T
            qT = persist.tile([DPC, S], F16)          # Q^T: [d', s]
            kT = persist.tile([DPC, S], F16)          # K^T: [d', s]
            v_nat = persist.tile([128, ST, 2 * (DK + 1)], F16)
            uT = persist.tile([DPC, S], F16)          # normalized attn out^T

            # x^T comes pre-transposed from the host; 8 chunks so the first
            # proj block's data lands fast. Weights go first.
            xt_r = xt_in.rearrange("(kt p) s -> p kt s", p=128)
            wq_sb = const.tile([128, KT, DPC], F16)
            wk_sb = const.tile([128, KT, DPC], F16)
            wv_sb = const.tile([128, KT, DPC], F16)
            nc.sync.dma_start(wq_sb[:], wq.rearrange("(kt p) d -> p kt d", p=128))
            nc.sync.dma_start(wk_sb[:], wk.rearrange("(kt p) d -> p kt d", p=128))
            # x^T chunk 0 split by kt across 8 DMA rings so block 0 lands fast
            for kt in range(KT):
                nc.sync.dma_start(xT[:, kt, 0:512], xt_r[:, kt, 0:512])
            nc.sync.dma_start(wv_sb[:], wv.rearrange("(kt p) d -> p kt d", p=128))
            bq_sb = const.tile([DPC, 1], F32)
            bk_sb = const.tile([DPC, 1], F32)
            bv_sb = const.tile([DPC, 1], F32)
            nc.sync.dma_start(bq_sb[:], bq[:])
            nc.sync.dma_start(bk_sb[:], bk[:])
            nc.sync.dma_start(bv_sb[:], bv[:])
            for jh in range(1, 8):
                for kth in range(2):
                    nc.sync.dma_start(
                        xT[:, kth * 4:(kth + 1) * 4, jh * 512:(jh + 1) * 512],
                        xt_r[:, kth * 4:(kth + 1) * 4, jh * 512:(jh + 1) * 512],
                    )
            wo_sb = const.tile([DPC, DIM], F16)
            nc.sync.dma_start(wo_sb[:], wo[:])

            # dense PE warmup: trips the HAM activity window to full
            # clock while the input DMA streams
            with tc.tile_pool(name="psumw", bufs=2, space="PSUM") as psumw:
                for _w in range(NWARM):
                    wt = psumw.tile([128, 128], F32, tag="warm")
                    nc.tensor.matmul(wt[:], ident[:], ident[:],
                                     start=True, stop=True)

            for st in range(ST):
                nc.vector.tensor_copy(v_nat[:, st, DK:DK + 1], ones_f[:])
                nc.vector.tensor_copy(v_nat[:, st, 2 * DK + 1:], ones_f[:])

            def attn_scores(t, i, psum):
                qsl = slice(t * QW, (t + 1) * QW)
                s_ps = psum.tile([128, 2 * QW], F32, tag="s", bufs=2)
                for h in range(HPC):
                    hp = h * DK
                    nc.tensor.matmul(
                        s_ps[:, h * QW:(h + 1) * QW],
                        kT[hp:hp + DK, i * 128:(i + 1) * 128],
                        qT[hp:hp + DK, qsl],
                        start=True, stop=True,
                    )
                return s_ps

            def attn_exp(s_ps, on_dve):
                if on_dve:
                    i16 = pexp.tile([128, 2 * QW], I16, tag="i16")
                    nc.vector.tensor_scalar(i16[:], s_ps[:], SCH_A, SCH_B,
                                            ALU.mult, ALU.add)
                    return [i16[:, h * QW:(h + 1) * QW].bitcast(F16)
                            for h in range(HPC)]
                p_sb = pexp.tile([128, 2 * QW], F16, tag="p")
                nc.scalar.activation(p_sb[:], s_ps[:], AF.Exp, scale=SCALE)
                return [p_sb[:, h * QW:(h + 1) * QW] for h in range(HPC)]

            def attn_av(i, u0, u1, p_slices):
                for h, u in ((0, u0), (1, u1)):
                    nc.tensor.matmul(
                        u[:],
                        v_nat[:, i, h * (DK + 1):(h + 1) * (DK + 1)],
                        p_slices[h],
                        start=(i == 0), stop=(i == ST - 1),
                    )

            def attn_iter(t, i, u0, u1, psum, on_dve):
                s_ps = attn_scores(t, i, psum)
                attn_av(i, u0, u1, attn_exp(s_ps, on_dve))

            def normalize(t, u0, u1, psum=None):
                """uT[h, qsl] = u[0:64] * recip(u[64]). u is copied to SBUF
                FIRST so the two PSUM u-banks free within ~1us of the last
                AV (the next stripe's accumulators reuse them). The
                reciprocal'd ones-row is broadcast to 64 partitions either
                by a C=1 PE outer-product against a ones column (fast,
                needs a free PSUM bank via the 'y' tag) or by a DRAM bounce
                (phase-0 pool has no spare bank). Head 1's result reaches
                partitions 64-127 via a gpsimd SBUF-SBUF DMA."""
                qsl = slice(t * QW, (t + 1) * QW)
                # head 1 first: its ush partition-shift DMA is the longest
                # leg of the chain, so issue it as early as possible
                for h, u in ((1, u1), (0, u0)):
                    uraw = work.tile([DK + 1, QW], F32, tag="uraw")
                    nc.vector.tensor_copy(uraw[:], u[:])
                    rd = dram.tile([1, QW], F32)
                    nc.sync.dma_start(rd[:], uraw[DK:DK + 1, :])
                    rb = work.tile([64, QW], F32, tag="rb")
                    nc.gpsimd.dma_start(
                        rb[:],
                        bass.AP(tensor=rd.tensor, offset=rd.offset,
                                ap=[[0, 64], [1, QW]]),
                    )
                    rec = work.tile([64, QW], F32, tag="rec")
                    nc.vector.reciprocal_approx_fast(rec[:], rb[:])
                    rec_ap = rec[:]
                    if h == 0:
                        nc.vector.tensor_mul(uT[0:DK, qsl], uraw[0:DK, :],
                                             rec_ap)
                    else:
                        ush = work.tile([DK, QW], F16, tag="ush")
                        nc.vector.tensor_mul(ush[:], uraw[0:DK, :], rec_ap)
                        nc.gpsimd.dma_start(uT[DK:2 * DK, qsl], ush[:])

            def out_proj_tile(q, psum_pool):
                # two half-tiles (1 PSUM bank each, double-buffered) so the
                # second matmul never waits on the first half's evacuation
                for m in range(DIM // 512):
                    yp = psum_pool.tile([128, 512], F32, tag="y", bufs=2)
                    nc.tensor.matmul(
                        yp[:],
                        uT[:, q * 128:(q + 1) * 128],
                        wo_sb[:, m * 512:(m + 1) * 512],
                        start=True, stop=True,
                    )
                    ysb = work.tile([128, 512], F16, tag="ysb", bufs=4)
                    nc.vector.tensor_copy(ysb[:], yp[:])
                    nc.sync.dma_start(
                        y[q * 128:(q + 1) * 128, m * 512:(m + 1) * 512],
                        ysb[:])

            with tc.tile_pool(name="psum12", bufs=1, space="PSUM") as psum:

                def proj_one(j, which):
                    """One of the Q/K/V projections for seq block j."""
                    sl = slice(j * 512, (j + 1) * 512)
                    w_sb, b_sb, dst = (
                        (wq_sb, bq_sb, qT),
                        (wk_sb, bk_sb, kT),
                        (wv_sb, bv_sb, None),
                    )[which]
                    pp = psum.tile([128, 512], F32, tag="proj", bufs=1)
                    for kt in range(KT):
                        nc.tensor.matmul(
                            pp[:], w_sb[:, kt, :], xT[:, kt, sl],
                            start=(kt == 0), stop=(kt == KT - 1),
                        )
                    if dst is not None:
                        nc.vector.tensor_scalar_add(dst[:, sl], pp[:], b_sb[:])
                    else:
                        vt = work.tile([128, 512], F16, tag="vt")
                        nc.vector.tensor_scalar_add(vt[:], pp[:], b_sb[:])
                        tpv = psum.tile([128, 512], F16, tag="tp", bufs=1)
                        for a in range(4):
                            nc.tensor.transpose(
                                tpv[:, a * 128:(a + 1) * 128],
                                vt[:, a * 128:(a + 1) * 128],
                                ident[:],
                            )
                        for a in range(4):
                            st = j * 4 + a
                            nc.vector.tensor_copy(
                                v_nat[:, st, 0:DK],
                                tpv[:, a * 128:a * 128 + DK],
                            )
                            nc.vector.tensor_copy(
                                v_nat[:, st, DK + 1:2 * DK + 1],
                                tpv[:, a * 128 + DK:(a + 1) * 128],
                            )

                def stripe_u_tiles():
                    u0 = psum.tile([DK + 1, QW], F32, tag="u0", bufs=1)
                    u1 = psum.tile([DK + 1, QW], F32, tag="u1", bufs=1)
                    return u0, u1

                # Stripe 0 woven through the projection loop. Group j-1's
                # four attention iterations are emitted BETWEEN block j's
                # K/V/Q projection chunks so their scores reach the PE early
                # and the ACT exp stream never starves behind a whole
                # projection block (that serialization cost ~44us of ACT
                # idle in the previous layout).
                u0, u1 = stripe_u_tiles()
                proj_one(0, 0), proj_one(0, 1), proj_one(0, 2)
                for j in range(1, KT + 1):
                    iters = list(range(4 * (j - 1), 4 * j))
                    sA = attn_scores(0, iters[0], psum)
                    sB = attn_scores(0, iters[1], psum)
                    pA = attn_exp(sA, False)
                    pB = attn_exp(sB, False)
                    if j <= KT - 1:
                        proj_one(j, 1)          # K(j)
                    attn_av(iters[0], u0, u1, pA)
                    sC = attn_scores(0, iters[2], psum)
                    pC = attn_exp(sC, False)
                    if j <= KT - 1:
                        proj_one(j, 2)          # V(j)
                    attn_av(iters[1], u0, u1, pB)
                    sD = attn_scores(0, iters[3], psum)
                    pD = attn_exp(sD, False)
                    if j <= KT - 1:
                        proj_one(j, 0)          # Q(j)
                    attn_av(iters[2], u0, u1, pC)
                    attn_av(iters[3], u0, u1, pD)
                normalize(0, u0, u1)

            # stripes 1-7 with the out-projection of the previous stripe
            # spread over this stripe's early iterations
            with tc.tile_pool(name="psum2b", bufs=1, space="PSUM") as psum:
                # Software-pipelined emission with AV lag 2: scores(i) enters
                # the in-order PE queue two iterations ahead of AV(i), so a
                # DVE Schraudolph exp can start the moment its scores land —
                # overlapping the ACT exp of the previous iteration instead
                # of serializing behind the AV/scores round trip.
                for t in range(1, NT):
                    u0, u1 = stripe_u_tiles()
                    pend = []
                    for i in range(ST):
                        s_ps = attn_scores(t, i, psum)
                        if i in OUTPROJ_AT:
                            out_proj_tile((t - 1) * 4 + OUTPROJ_AT.index(i),
                                          psum)
                        if len(pend) == 2:
                            attn_av(*pend.pop(0))
                        pend.append((i, u0, u1,
                                     attn_exp(s_ps, i in DVE_SET)))
                    for item in pend:
                        attn_av(*item)
                    normalize(t, u0, u1, psum)
                for q in range((NT - 1) * 4, NT * 4):
                    out_proj_tile(q, psum)

    nc.finalize()
    return nc


_NC_CACHE = None


def _get_nc():
    global _NC_CACHE
    if _NC_CACHE is None:
        _NC_CACHE = build_bass()
    return _NC_CACHE


def kernel(x, Wq, bq, Wk, bk, Wv, bv, Wo, bo, _want_results=False, **run_kwargs):
    xt_host = np.ascontiguousarray(
        np.asarray(x, dtype=np.float32).reshape(S, DIM).T).astype(np.float16)
    Wq = np.asarray(Wq, dtype=np.float32).astype(np.float16)
    Wk = np.asarray(Wk, dtype=np.float32).astype(np.float16)
    Wv = np.asarray(Wv, dtype=np.float32).astype(np.float16)
    Wo = np.asarray(Wo, dtype=np.float32).astype(np.float16)
    bq = np.asarray(bq, dtype=np.float32)
    bk = np.asarray(bk, dtype=np.float32)
    bv = np.asarray(bv, dtype=np.float32)
    bo = np.asarray(bo, dtype=np.float32)

    nc = _get_nc()
    in_maps = []
    for c in range(NCORES):
        sl = slice(c * DPC, (c + 1) * DPC)
        in_maps.append({
            "xt": xt_host,
            "wq": np.ascontiguousarray(Wq[:, sl]),
            "wk": np.ascontiguousarray(Wk[:, sl]),
            "wv": np.ascontiguousarray(Wv[:, sl]),
            "bq": np.ascontiguousarray(bq[sl]).reshape(DPC, 1),
            "bk": np.ascontiguousarray(bk[sl]).reshape(DPC, 1),
            "bv": np.ascontiguousarray(bv[sl]).reshape(DPC, 1),
            "wo": np.ascontiguousarray(Wo[sl, :]),
        })
    res = run_bass_kernel_spmd(nc, in_maps, core_ids=list(range(NCORES)),
                               **run_kwargs)
    out = np.zeros((S, DIM), dtype=np.float64)
    for c in range(NCORES):
        out += res.results[c]["y"].astype(np.float64)
    out += bo.astype(np.float64)
    out = out.astype(np.float32).reshape(1, S, DIM)
    if _want_results:
        return out, res
    return out
